# revision 10
# baseline (speedup 1.0000x reference)
"""ClusterAttn Trainium2 kernel (Bass/Tile), 8-way data parallel over batch.

Full inputs in, full outputs out. Internally:
  - batch B=32 is split 4-per-core across 8 NeuronCores (pure DP).
  - BN1 is folded into cluster_weights (+W_ga appended) on the host;
    BN2 folded into a per-cluster scale + bias matrix; Wq pre-transposed.
  - attention is re-associated: scores = x @ (Wq @ k^T), out = attn @ (v @ Wp2).
  - all matmuls run as float32r (full PE rate at moving-dim >= 256).
"""

from contextlib import ExitStack

import numpy as np

import concourse.bass as bass
import concourse.bacc as bacc
import concourse.tile as tile
import concourse.mybir as mybir
from concourse import bass_utils
from concourse.masks import make_identity

dt = mybir.dt
AF = mybir.ActivationFunctionType
ALU = mybir.AluOpType

EPS = 1e-5
N_CORES = 8
B, S, D = 32, 1024, 768
E, G, C, P = 2, 8, 64, 384
EF = E * D            # 1536
GC = G * C            # 512
GFS = EF // G         # 192
NB = B // N_CORES     # batches per core
NT = S // 128         # token tiles per batch
F32 = dt.float32
F32R = dt.float32r


def _r(ap):
    """View an fp32 AP as float32r for the tensor engine."""
    return ap.bitcast(F32R)


def _f(ap):
    """View a float32r AP as fp32 for vector/scalar engines."""
    return ap.bitcast(F32)


def build_program(flags):
    has_bexp, has_bq, has_bkv, has_bp2 = flags
    nc = bacc.Bacc(
        "TRN2",
        debug=False,
        enable_asserts=False,
        num_devices=N_CORES,
    )

    x_d = nc.dram_tensor("x", (NB, S, D), F32R, kind="ExternalInput").ap()
    out_d = nc.dram_tensor("out", (NB, S, D), F32, kind="ExternalOutput").ap()
    wexp_d = nc.dram_tensor("wexp", (D, EF), F32R, kind="ExternalInput").ap()
    cwf_d = nc.dram_tensor("cwf", (EF, GC + G), F32R, kind="ExternalInput").ap()
    bias1_d = nc.dram_tensor("bias1", (GC + G,), F32, kind="ExternalInput").ap()
    wproj_d = nc.dram_tensor("wproj", (GFS, D), F32R, kind="ExternalInput").ap()
    s2_d = nc.dram_tensor("s2", (C, 1), F32, kind="ExternalInput").ap()
    bias2_d = nc.dram_tensor("bias2", (C, D), F32, kind="ExternalInput").ap()
    wkv_d = nc.dram_tensor("wkv", (D, 2 * P), F32R, kind="ExternalInput").ap()
    wqT_d = nc.dram_tensor("wqT", (P, D), F32R, kind="ExternalInput").ap()
    wp2_d = nc.dram_tensor("wp2", (P, D), F32R, kind="ExternalInput").ap()
    bexp_d = bq_d = bkv_d = bp2_d = None
    if has_bexp:
        bexp_d = nc.dram_tensor("bexp", (EF,), F32, kind="ExternalInput").ap()
    if has_bq:
        bq_d = nc.dram_tensor("bqT", (P, 1), F32R, kind="ExternalInput").ap()
    if has_bkv:
        bkv_d = nc.dram_tensor("bkv", (2 * P,), F32, kind="ExternalInput").ap()
    if has_bp2:
        bp2_d = nc.dram_tensor("bp2", (D,), F32, kind="ExternalInput").ap()

    with tile.TileContext(nc) as tc, ExitStack() as ctx:
        # ---------------- pools ----------------
        const = ctx.enter_context(tc.tile_pool(name="const", bufs=1))
        mid = ctx.enter_context(tc.tile_pool(name="mid", bufs=1))
        p_x = ctx.enter_context(tc.tile_pool(name="p_x", bufs=3))
        p_xt = ctx.enter_context(tc.tile_pool(name="p_xt", bufs=3))
        p_fea = ctx.enter_context(tc.tile_pool(name="p_fea", bufs=2))
        p_feat = ctx.enter_context(tc.tile_pool(name="p_feat", bufs=2))
        p_act = ctx.enter_context(tc.tile_pool(name="p_act", bufs=2))
        p_sm = ctx.enter_context(tc.tile_pool(name="p_sm", bufs=4))
        p_out = ctx.enter_context(tc.tile_pool(name="p_out", bufs=2))
        ps_t = ctx.enter_context(tc.tile_pool(name="ps_t", bufs=2, space="PSUM"))
        ps_mm = ctx.enter_context(tc.tile_pool(name="ps_mm", bufs=2, space="PSUM"))
        ps_c = ctx.enter_context(tc.tile_pool(name="ps_c", bufs=4, space="PSUM"))

        # ---------------- constants ----------------
        ident_f = const.tile([128, 128], F32)
        make_identity(nc, ident_f[:])
        ident = const.tile([128, 128], F32R)
        nc.vector.tensor_copy(ident[:], ident_f[:])

        wexp_sb = const.tile([128, 6, EF], F32R)
        wexp_r = wexp_d.rearrange("(ko ki) n -> ki ko n", ki=128)
        cwf_sb = const.tile([128, 12, GC + G], F32R)
        cwf_r = cwf_d.rearrange("(ko ki) n -> ki ko n", ki=128)
        nc.sync.dma_start(wexp_sb[:, :, 0:512], wexp_r[:, :, 0:512])
        nc.sync.dma_start(cwf_sb[:, :, 0:260], cwf_r[:, :, 0:260])
        nc.sync.dma_start(wexp_sb[:, :, 512:1024], wexp_r[:, :, 512:1024])
        nc.sync.dma_start(wexp_sb[:, :, 1024:1536], wexp_r[:, :, 1024:1536])
        nc.sync.dma_start(cwf_sb[:, :, 260:520], cwf_r[:, :, 260:520])
        bias1_sb = const.tile([128, GC + G], F32)
        nc.gpsimd.dma_start(bias1_sb[:], bias1_d.partition_broadcast(128))
        wproj_sb = const.tile([128, 2, D], F32R)
        nc.sync.dma_start(wproj_sb[:, 0, :], wproj_d[0:128, :])
        nc.sync.dma_start(wproj_sb[0:64, 1, :], wproj_d[128:GFS, :])
        s2_sb = const.tile([C, 1], F32)
        nc.sync.dma_start(s2_sb[:], s2_d)
        bias2_sb = const.tile([C, D], F32)
        nc.sync.dma_start(bias2_sb[:], bias2_d)
        wkv_sb = const.tile([128, 6, 2 * P], F32R)
        nc.sync.dma_start(wkv_sb[:], wkv_d.rearrange("(ko ki) n -> ki ko n", ki=128))
        wqT_sb = const.tile([128, 3, D], F32R)
        nc.sync.dma_start(wqT_sb[:], wqT_d.rearrange("(ko ki) n -> ki ko n", ki=128))
        wp2_sb = const.tile([128, 3, D], F32R)
        nc.sync.dma_start(wp2_sb[:], wp2_d.rearrange("(ko ki) n -> ki ko n", ki=128))
        if has_bexp:
            bexp_sb = const.tile([128, EF], F32)
            nc.gpsimd.dma_start(bexp_sb[:], bexp_d.partition_broadcast(128))
        if has_bq:
            bq_sb = const.tile([128, 3, 1], F32R)
            nc.sync.dma_start(bq_sb[:], bq_d.rearrange("(ko ki) n -> ki ko n", ki=128))
            ones_sb = const.tile([1, 128], F32R)
            nc.vector.memset(ones_sb[:], 1.0)
        if has_bkv:
            bkv_sb = const.tile([C, 2 * P], F32)
            nc.gpsimd.dma_start(bkv_sb[:], bkv_d.partition_broadcast(C))
        if has_bp2:
            bp2_sb = const.tile([128, D], F32)
            nc.gpsimd.dma_start(bp2_sb[:], bp2_d.partition_broadcast(128))

        def transpose_to(out_ps, in_ap, start=True, stop=True):
            """PE transpose of fp32 in_ap -> float32r psum tile slice."""
            kp = in_ap.partition_size()
            nc.tensor.matmul(
                out_ps,
                _r(in_ap),
                _r(ident[0:kp, 0:kp]),
                is_transpose=True,
                start=start,
                stop=stop,
                skip_group_check=True,
            )

        def load_xt(b, t):
            """DMA one x token-tile and produce xT [128, 6, 128] in SBUF."""
            xx = p_x.tile([128, D], F32R, tag="x")
            nc.sync.dma_start(xx[:], x_d[b, t * 128:(t + 1) * 128, :])
            xt = p_xt.tile([128, 6, 128], F32R, tag="xt")
            tp0 = ps_t.tile([128, 512], F32R, tag="t")
            for i in range(4):
                transpose_to(tp0[:, i * 128:(i + 1) * 128], xx[:, i * 128:(i + 1) * 128],
                             start=(i == 0), stop=(i == 3))
            nc.vector.tensor_copy(xt[:, 0:4, :].rearrange("p a b -> p (a b)"), _f(tp0[:]))
            tp1 = ps_t.tile([128, 512], F32R, tag="t")
            for i in range(2):
                transpose_to(tp1[:, i * 128:(i + 1) * 128], xx[:, (4 + i) * 128:(5 + i) * 128],
                             start=(i == 0), stop=(i == 1))
            nc.vector.tensor_copy(xt[:, 4:6, :].rearrange("p a b -> p (a b)"),
                                  _f(tp1[:, 0:256]))
            return xt

        cent_store = {}
        inv_sqrt_p = float(1.0 / np.sqrt(np.float32(P)))

        def pass1_tile(b, t):
            if b not in cent_store:
                cent_store[b] = [
                    ps_c.tile([128, 384], F32, tag="cent", name=f"cent{b}_{mi}")
                    for mi in range(4)]
            cent_ps = cent_store[b]
            if True:
                xt = load_xt(b, t)

                # fea = x @ W_exp (+ b_exp)   [128 tok, 1536]
                fea = p_fea.tile([128, EF], F32R, tag="fea")
                for n3 in range(3):
                    fp = ps_mm.tile([128, 512], F32, tag="mm")
                    for k in range(6):
                        nc.tensor.matmul(
                            fp[:], _r(xt[:, k, :]),
                            _r(wexp_sb[:, k, n3 * 512:(n3 + 1) * 512]),
                            start=(k == 0), stop=(k == 5))
                    dst = fea[:, n3 * 512:(n3 + 1) * 512]
                    if has_bexp:
                        nc.vector.tensor_add(dst, fp[:], bexp_sb[:, n3 * 512:(n3 + 1) * 512])
                    else:
                        nc.scalar.copy(dst, fp[:])

                # feaT [128 ef, 12, 128 tok]
                feat = p_feat.tile([128, 12, 128], F32R, tag="feat")
                for grp in range(3):
                    tp = ps_t.tile([128, 512], F32R, tag="t")
                    for i in range(4):
                        m = grp * 4 + i
                        transpose_to(tp[:, i * 128:(i + 1) * 128],
                                     fea[:, m * 128:(m + 1) * 128],
                                     start=(i == 0), stop=(i == 3))
                    nc.vector.tensor_copy(
                        feat[:, grp * 4:(grp + 1) * 4, :].rearrange("p a b -> p (a b)"),
                        _f(tp[:]))

                # act_pre = fea @ cw_f + bias1   [128 tok, 520]
                act = p_act.tile([128, GC + G], F32, tag="act")
                for n2 in range(2):
                    apm = ps_mm.tile([128, 260], F32, tag="mm")
                    for k2 in range(12):
                        nc.tensor.matmul(
                            apm[:], _r(feat[:, k2, :]),
                            _r(cwf_sb[:, k2, n2 * 260:(n2 + 1) * 260]),
                            start=(k2 == 0), stop=(k2 == 11))
                    nc.vector.tensor_add(act[:, n2 * 260:(n2 + 1) * 260], apm[:],
                                         bias1_sb[:, n2 * 260:(n2 + 1) * 260])

                # grouped softmax * sigmoid gate
                e = p_act.tile([128, GC], F32, tag="e")
                nc.scalar.activation(e[:], act[:, 0:GC], AF.Exp)
                ssum = p_sm.tile([128, G], F32, tag="ssum")
                nc.vector.reduce_sum(ssum[:], e[:].rearrange("p (g c) -> p g c", g=G),
                                     axis=mybir.AxisListType.X)
                eneg = p_sm.tile([128, G], F32, tag="eneg")
                nc.scalar.activation(eneg[:], act[:, GC:GC + G], AF.Exp, scale=-1.0)
                nc.vector.tensor_scalar_add(eneg[:], eneg[:], 1.0)
                ga = p_sm.tile([128, G], F32, tag="ga")
                nc.vector.reciprocal(ga[:], eneg[:])
                rs = p_sm.tile([128, G], F32, tag="rs")
                nc.vector.reciprocal(rs[:], ssum[:])
                nc.vector.tensor_mul(rs[:], rs[:], ga[:])
                actf = p_act.tile([128, GC], F32R, tag="actf")
                nc.vector.tensor_tensor(
                    out=actf[:].rearrange("p (g c) -> p g c", g=G),
                    in0=e[:].rearrange("p (g c) -> p g c", g=G),
                    in1=rs[:].unsqueeze(2).broadcast_to((128, G, C)),
                    op=ALU.mult)

                # cent accumulation (diagonal blocks)
                for mi in range(4):
                    nc.tensor.matmul(
                        cent_ps[mi][:], _r(actf[:, mi * 128:(mi + 1) * 128]),
                        _r(fea[:, mi * 384:(mi + 1) * 384]),
                        start=(t == 0), stop=(t == NT - 1))

        def mid_phase(b):
            cent_ps = cent_store[b]
            cent = mid.tile([C, GFS], F32R, tag="cent_acc")
            nc.vector.tensor_copy(cent[:], cent_ps[0][0:64, 0:192])
            nc.vector.tensor_add(cent[:], cent[:], cent_ps[0][64:128, 192:384])
            for mi in range(1, 4):
                nc.vector.tensor_add(cent[:], cent[:], cent_ps[mi][0:64, 0:192])
                nc.vector.tensor_add(cent[:], cent[:], cent_ps[mi][64:128, 192:384])

            # centT [192, 64] as [128, 2, 64]
            centT = mid.tile([128, 2, C], F32R, tag="centT")
            ctp = ps_t.tile([128, 512], F32R, tag="t")
            transpose_to(ctp[:, 0:64], cent[:, 0:128])
            transpose_to(ctp[0:64, 64:128], cent[:, 128:192])
            nc.vector.tensor_copy(centT[:, 0, :], _f(ctp[:, 0:64]))
            nc.vector.tensor_copy(centT[0:64, 1, :], _f(ctp[0:64, 64:128]))

            # nc2 = BN2(cent @ W_proj + b_proj)  [64, 768]
            nc2 = mid.tile([C, D], F32R, tag="nc2")
            for ng, (n0, nn) in enumerate(((0, 512), (512, 256))):
                np_ps = ps_mm.tile([128, 512], F32, tag="mm")
                nc.tensor.matmul(np_ps[0:C, 0:nn], _r(centT[:, 0, :]),
                                 _r(wproj_sb[:, 0, n0:n0 + nn]), start=True, stop=False)
                nc.tensor.matmul(np_ps[0:C, 0:nn], _r(centT[0:64, 1, :]),
                                 _r(wproj_sb[0:64, 1, n0:n0 + nn]), start=False, stop=True)
                nc.vector.scalar_tensor_tensor(
                    out=nc2[:, n0:n0 + nn], in0=np_ps[0:C, 0:nn], scalar=s2_sb[:, 0:1],
                    in1=bias2_sb[:, n0:n0 + nn], op0=ALU.mult, op1=ALU.add)

            # nc2T [768, 64] as [128, 6, 64]
            nc2T = mid.tile([128, 6, C], F32R, tag="nc2T")
            for grp in range(2):
                ntp = ps_t.tile([128, 512], F32R, tag="t")
                for i in range(3):
                    transpose_to(ntp[:, i * 64:(i + 1) * 64],
                                 nc2[:, (grp * 3 + i) * 128:(grp * 3 + i + 1) * 128],
                                 start=(i == 0), stop=(i == 2))
                nc.vector.tensor_copy(
                    nc2T[:, grp * 3:(grp + 1) * 3, :].rearrange("p a b -> p (a b)"),
                    _f(ntp[:, 0:192]))

            # kv = nc2 @ Wkv (+ bkv)   [64, 768]
            kv = mid.tile([C, 2 * P], F32R, tag="kv")
            for ng, (n0, nn) in enumerate(((0, 512), (512, 256))):
                kv_ps = ps_mm.tile([128, 512], F32, tag="mm")
                for k in range(6):
                    nc.tensor.matmul(kv_ps[0:C, 0:nn], _r(nc2T[:, k, :]),
                                     _r(wkv_sb[:, k, n0:n0 + nn]),
                                     start=(k == 0), stop=(k == 5))
                if has_bkv:
                    nc.vector.tensor_add(kv[:, n0:n0 + nn], kv_ps[0:C, 0:nn],
                                         bkv_sb[:, n0:n0 + nn])
                else:
                    nc.scalar.copy(kv[:, n0:n0 + nn], kv_ps[0:C, 0:nn])

            # kT, vT  [128, 3, 64]
            kT = mid.tile([128, 3, C], F32R, tag="kT")
            vT = mid.tile([128, 3, C], F32R, tag="vT")
            ktp = ps_t.tile([128, 512], F32R, tag="t")
            for i in range(3):
                transpose_to(ktp[:, i * 64:(i + 1) * 64], kv[:, i * 128:(i + 1) * 128],
                             start=(i == 0), stop=(i == 2))
            nc.vector.tensor_copy(kT[:].rearrange("p a b -> p (a b)"), _f(ktp[:, 0:192]))
            vtp = ps_t.tile([128, 512], F32R, tag="t")
            for i in range(3):
                transpose_to(vtp[:, i * 64:(i + 1) * 64],
                             kv[:, P + i * 128:P + (i + 1) * 128],
                             start=(i == 0), stop=(i == 2))
            nc.vector.tensor_copy(vT[:].rearrange("p a b -> p (a b)"), _f(vtp[:, 0:192]))

            # wqk = Wq @ k^T  [768, 64] as [128, 6, 64]
            wqk = mid.tile([128, 6, C], F32R, tag="wqk")
            for m in range(6):
                wq_ps = ps_t.tile([128, 512], F32, tag="t")
                for k3 in range(3):
                    nc.tensor.matmul(wq_ps[:, 0:C], _r(wqT_sb[:, k3, m * 128:(m + 1) * 128]),
                                     _r(kT[:, k3, :]), start=(k3 == 0), stop=(k3 == 2))
                nc.scalar.copy(wqk[:, m, :], wq_ps[:, 0:C])
            if has_bq:
                bc_ps = ps_t.tile([128, 512], F32, tag="t")
                for k3 in range(3):
                    nc.tensor.matmul(bc_ps[0:1, 0:C], _r(bq_sb[:, k3, :]),
                                     _r(kT[:, k3, :]), start=(k3 == 0), stop=(k3 == 2))
                bias_c = mid.tile([1, C], F32R, tag="bias_c")
                nc.scalar.copy(bias_c[:], bc_ps[0:1, 0:C])

            if not has_bq:
                bias_c = None

            # vW = v @ Wp2  [64, 768]
            vw = mid.tile([C, D], F32R, tag="vw")
            for ng, (n0, nn) in enumerate(((0, 512), (512, 256))):
                vw_ps = ps_mm.tile([128, 512], F32, tag="mm")
                for k3 in range(3):
                    nc.tensor.matmul(vw_ps[0:C, 0:nn], _r(vT[:, k3, :]),
                                     _r(wp2_sb[:, k3, n0:n0 + nn]),
                                     start=(k3 == 0), stop=(k3 == 2))
                nc.scalar.copy(vw[:, n0:n0 + nn], vw_ps[0:C, 0:nn])

            return {"wqk": wqk, "vw": vw,
                    "bias_c": bias_c}

        def pass2_tile(b, t, mt):
            wqk, vw, bias_c = mt["wqk"], mt["vw"], mt["bias_c"]
            if True:
                xt = load_xt(b, t)
                sc_ps = ps_t.tile([128, 512], F32, tag="t")
                for k in range(6):
                    nc.tensor.matmul(sc_ps[:, 0:C], _r(xt[:, k, :]), _r(wqk[:, k, :]),
                                     start=(k == 0), stop=(k == 5 and not has_bq),
                                     skip_group_check=True)
                if has_bq:
                    nc.tensor.matmul(sc_ps[:, 0:C], _r(ones_sb[:]), _r(bias_c[:]),
                                     start=False, stop=True, skip_group_check=True)

                e_att = p_sm.tile([128, C], F32, tag="e_att")
                ssum_a = p_sm.tile([128, 1], F32, tag="ssum_a")
                nc.scalar.activation(e_att[:], sc_ps[:, 0:C], AF.Exp,
                                     scale=inv_sqrt_p, accum_out=ssum_a[:])
                rs_a = p_sm.tile([128, 1], F32, tag="rs_a")
                nc.vector.reciprocal(rs_a[:], ssum_a[:])
                attn = p_sm.tile([128, C], F32R, tag="attn")
                nc.vector.tensor_scalar_mul(attn[:], e_att[:], rs_a[:])

                at_ps = ps_t.tile([128, 512], F32R, tag="t")
                transpose_to(at_ps[0:C, 0:128], attn[:])
                attnT = p_sm.tile([C, 128], F32R, tag="attnT")
                nc.vector.tensor_copy(attnT[:], _f(at_ps[0:C, 0:128]))

                outt = p_out.tile([128, D], F32, tag="outt")
                for ng, (n0, nn) in enumerate(((0, 512), (512, 256))):
                    fo_ps = ps_mm.tile([128, 512], F32, tag="mm")
                    nc.tensor.matmul(fo_ps[:, 0:nn], _r(attnT[:]),
                                     _r(vw[:, n0:n0 + nn]), start=True, stop=True)
                    if has_bp2:
                        nc.vector.tensor_add(outt[:, n0:n0 + nn], fo_ps[:, 0:nn],
                                             bp2_sb[:, n0:n0 + nn])
                    else:
                        nc.scalar.copy(outt[:, n0:n0 + nn], fo_ps[:, 0:nn])
                nc.sync.dma_start(out_d[b, t * 128:(t + 1) * 128, :], outt[:])

        # ---------------- interleaved emission ----------------
        # pass1 of batch b+1 overlaps mid+pass2 of batch b so the PE never
        # idles long enough to re-trigger the HAM throttle.
        for t in range(NT):
            pass1_tile(0, t)
        for b in range(NB):
            nxt = b + 1 if b + 1 < NB else None
            if nxt is not None:
                pass1_tile(nxt, 0)
                pass1_tile(nxt, 1)
            mt = mid_phase(b)
            for t in range(NT):
                pass2_tile(b, t, mt)
                if nxt is not None and t + 2 < NT:
                    pass1_tile(nxt, t + 2)

    nc.compile()
    return nc


_PROGRAM_CACHE = {}


def _prep(inputs):
    """Host-side folds. Returns (flags, in_map_common)."""
    f32 = np.float32
    g = {k: np.ascontiguousarray(np.asarray(v, dtype=f32)) for k, v in inputs.items()}
    s1 = g["bn1_g"] / np.sqrt(g["bn1_v"] + f32(EPS))
    cwf = np.concatenate([g["cluster_weights"] * s1[None, :], g["W_ga"]], axis=1)
    bias1 = np.concatenate([g["bn1_b"] - g["bn1_m"] * s1, g["b_ga"]])
    s2 = g["bn2_g"] / np.sqrt(g["bn2_v"] + f32(EPS))
    bias2 = (g["b_proj"][None, :] - g["bn2_m"][:, None]) * s2[:, None] + g["bn2_b"][:, None]
    flags = (
        bool(np.any(g["b_exp"])),
        bool(np.any(g["bq"])),
        bool(np.any(g["bkv"])),
        bool(np.any(g["bp2"])),
    )
    common = {
        "wexp": g["W_exp"],
        "cwf": np.ascontiguousarray(cwf),
        "bias1": np.ascontiguousarray(bias1),
        "wproj": g["W_proj"],
        "s2": np.ascontiguousarray(s2.reshape(C, 1)),
        "bias2": np.ascontiguousarray(bias2),
        "wkv": g["Wkv"],
        "wqT": np.ascontiguousarray(g["Wq"].T),
        "wp2": g["Wp2"],
    }
    if flags[0]:
        common["bexp"] = g["b_exp"]
    if flags[1]:
        common["bqT"] = np.ascontiguousarray(g["bq"].reshape(P, 1))
    if flags[2]:
        common["bkv"] = g["bkv"]
    if flags[3]:
        common["bp2"] = g["bp2"]
    return flags, common, g["x"]


def run(inputs, trace=False):
    flags, common, x = _prep(inputs)
    if flags not in _PROGRAM_CACHE:
        _PROGRAM_CACHE[flags] = build_program(flags)
    nc = _PROGRAM_CACHE[flags]
    in_maps = []
    for c in range(N_CORES):
        m = dict(common)
        m["x"] = np.ascontiguousarray(x[c * NB:(c + 1) * NB])
        in_maps.append(m)
    res = bass_utils.run_bass_kernel_spmd(
        nc, in_maps, core_ids=list(range(N_CORES)), trace=trace)
    out = np.concatenate([r["out"] for r in res.results], axis=0)
    return out, res


def kernel(**inputs):
    out, _ = run(inputs, trace=False)
    return out


# revision 13
# speedup vs baseline: 1.0420x; 1.0420x over previous
"""ClusterAttn Trainium2 kernel (Bass/Tile), 8-way data parallel over batch.

Full inputs in, full outputs out. Internally:
  - batch B=32 is split 4-per-core across 8 NeuronCores (pure DP).
  - BN1 is folded into cluster_weights (+W_ga appended) on the host;
    BN2 folded into a per-cluster scale + bias matrix; Wq pre-transposed.
  - attention is re-associated: scores = x @ (Wq @ k^T), out = attn @ (v @ Wp2).
  - all matmuls run as float32r (full PE rate at moving-dim >= 256).
"""

from contextlib import ExitStack

import numpy as np

import concourse.bass as bass
import concourse.bacc as bacc
import concourse.tile as tile
import concourse.mybir as mybir
from concourse import bass_utils
from concourse.masks import make_identity

dt = mybir.dt
AF = mybir.ActivationFunctionType
ALU = mybir.AluOpType

EPS = 1e-5
N_CORES = 8
B, S, D = 32, 1024, 768
E, G, C, P = 2, 8, 64, 384
EF = E * D            # 1536
GC = G * C            # 512
GFS = EF // G         # 192
NB = B // N_CORES     # batches per core
NT = S // 128         # token tiles per batch
F32 = dt.float32
F32R = dt.float32r


def _r(ap):
    """View an fp32 AP as float32r for the tensor engine."""
    return ap.bitcast(F32R)


def _f(ap):
    """View a float32r AP as fp32 for vector/scalar engines."""
    return ap.bitcast(F32)


def build_program(flags):
    has_bexp, has_bq, has_bkv, has_bp2 = flags
    nc = bacc.Bacc(
        "TRN2",
        debug=False,
        enable_asserts=False,
        num_devices=N_CORES,
    )

    x_d = nc.dram_tensor("x", (NB, S, D), F32R, kind="ExternalInput").ap()
    out_d = nc.dram_tensor("out", (NB, S, D), F32, kind="ExternalOutput").ap()
    wexp_d = nc.dram_tensor("wexp", (D, EF), F32R, kind="ExternalInput").ap()
    cwf_d = nc.dram_tensor("cwf", (EF, GC + G), F32R, kind="ExternalInput").ap()
    bias1_d = nc.dram_tensor("bias1", (GC + G,), F32, kind="ExternalInput").ap()
    wproj_d = nc.dram_tensor("wproj", (GFS, D), F32R, kind="ExternalInput").ap()
    s2_d = nc.dram_tensor("s2", (C, 1), F32, kind="ExternalInput").ap()
    bias2_d = nc.dram_tensor("bias2", (C, D), F32, kind="ExternalInput").ap()
    wkv_d = nc.dram_tensor("wkv", (D, 2 * P), F32R, kind="ExternalInput").ap()
    wqT_d = nc.dram_tensor("wqT", (P, D), F32R, kind="ExternalInput").ap()
    wp2_d = nc.dram_tensor("wp2", (P, D), F32R, kind="ExternalInput").ap()
    bexp_d = bq_d = bkv_d = bp2_d = None
    if has_bexp:
        bexp_d = nc.dram_tensor("bexp", (EF,), F32, kind="ExternalInput").ap()
    if has_bq:
        bq_d = nc.dram_tensor("bqT", (P, 1), F32R, kind="ExternalInput").ap()
    if has_bkv:
        bkv_d = nc.dram_tensor("bkv", (2 * P,), F32, kind="ExternalInput").ap()
    if has_bp2:
        bp2_d = nc.dram_tensor("bp2", (D,), F32, kind="ExternalInput").ap()

    with tile.TileContext(nc) as tc, ExitStack() as ctx:
        # ---------------- pools ----------------
        const = ctx.enter_context(tc.tile_pool(name="const", bufs=1))
        mid = ctx.enter_context(tc.tile_pool(name="mid", bufs=1))
        p_x = ctx.enter_context(tc.tile_pool(name="p_x", bufs=3))
        p_xt = ctx.enter_context(tc.tile_pool(name="p_xt", bufs=3))
        p_fea = ctx.enter_context(tc.tile_pool(name="p_fea", bufs=2))
        p_feat = ctx.enter_context(tc.tile_pool(name="p_feat", bufs=2))
        p_act = ctx.enter_context(tc.tile_pool(name="p_act", bufs=2))
        p_sm = ctx.enter_context(tc.tile_pool(name="p_sm", bufs=4))
        p_out = ctx.enter_context(tc.tile_pool(name="p_out", bufs=2))
        p_cacc = ctx.enter_context(tc.tile_pool(name="p_cacc", bufs=2))
        ps_t = ctx.enter_context(tc.tile_pool(name="ps_t", bufs=3, space="PSUM"))
        ps_mm = ctx.enter_context(tc.tile_pool(name="ps_mm", bufs=3, space="PSUM"))
        ps_c = ctx.enter_context(tc.tile_pool(name="ps_c", bufs=2, space="PSUM"))

        # ---------------- constants ----------------
        # (x prefetch is emitted first, inside the emission section below,
        #  via forward-declared pool; DMA order follows program order.)
        x_pref = {}
        for _pt in range(2):
            _xx = p_x.tile([128, D], F32R, tag="x", name=f"xpf0_{_pt}")
            nc.sync.dma_start(_xx[:], x_d[0, _pt * 128:(_pt + 1) * 128, :])
            x_pref[(0, _pt)] = _xx

        ident_f = const.tile([128, 128], F32)
        make_identity(nc, ident_f[:])
        ident = const.tile([128, 128], F32R)
        nc.vector.tensor_copy(ident[:], ident_f[:])

        wexp_sb = const.tile([128, 6, EF], F32R)
        wexp_r = wexp_d.rearrange("(ko ki) n -> ki ko n", ki=128)
        cwf_sb = const.tile([128, 12, GC + G], F32R)
        cwf_r = cwf_d.rearrange("(ko ki) n -> ki ko n", ki=128)
        nc.sync.dma_start(wexp_sb[:, :, 0:512], wexp_r[:, :, 0:512])
        nc.sync.dma_start(cwf_sb[:, :, 0:260], cwf_r[:, :, 0:260])
        bias1_sb = const.tile([128, GC + G], F32)
        nc.gpsimd.dma_start(bias1_sb[:], bias1_d.partition_broadcast(128))
        wproj_sb = const.tile([128, 2, D], F32R)
        s2_sb = const.tile([C, 1], F32)
        nc.sync.dma_start(s2_sb[:], s2_d)
        bias2_sb = const.tile([C, D], F32)
        nc.sync.dma_start(bias2_sb[:], bias2_d)
        wkv_sb = const.tile([128, 6, 2 * P], F32R)
        wqT_sb = const.tile([128, 3, D], F32R)
        wp2_sb = const.tile([128, 3, D], F32R)

        nc.sync.dma_start(wexp_sb[:, :, 512:1024], wexp_r[:, :, 512:1024])
        nc.sync.dma_start(cwf_sb[:, :, 260:520], cwf_r[:, :, 260:520])
        nc.sync.dma_start(wexp_sb[:, :, 1024:1536], wexp_r[:, :, 1024:1536])
        nc.sync.dma_start(wproj_sb[:, 0, :], wproj_d[0:128, :])
        nc.sync.dma_start(wproj_sb[0:64, 1, :], wproj_d[128:GFS, :])
        nc.sync.dma_start(wkv_sb[:], wkv_d.rearrange("(ko ki) n -> ki ko n", ki=128))
        nc.sync.dma_start(wqT_sb[:], wqT_d.rearrange("(ko ki) n -> ki ko n", ki=128))
        nc.sync.dma_start(wp2_sb[:], wp2_d.rearrange("(ko ki) n -> ki ko n", ki=128))
        if has_bexp:
            bexp_sb = const.tile([128, EF], F32)
            nc.gpsimd.dma_start(bexp_sb[:], bexp_d.partition_broadcast(128))
        if has_bq:
            bq_sb = const.tile([128, 3, 1], F32R)
            nc.sync.dma_start(bq_sb[:], bq_d.rearrange("(ko ki) n -> ki ko n", ki=128))
            ones_sb = const.tile([1, 128], F32R)
            nc.vector.memset(ones_sb[:], 1.0)
        if has_bkv:
            bkv_sb = const.tile([C, 2 * P], F32)
            nc.gpsimd.dma_start(bkv_sb[:], bkv_d.partition_broadcast(C))
        if has_bp2:
            bp2_sb = const.tile([128, D], F32)
            nc.gpsimd.dma_start(bp2_sb[:], bp2_d.partition_broadcast(128))

        def transpose_to(out_ps, in_ap, start=True, stop=True):
            """PE transpose of fp32 in_ap -> float32r psum tile slice."""
            kp = in_ap.partition_size()
            nc.tensor.matmul(
                out_ps,
                _r(in_ap),
                _r(ident[0:kp, 0:kp]),
                is_transpose=True,
                start=start,
                stop=stop,
                skip_group_check=True,
            )

        def load_xt(b, t):
            """DMA one x token-tile and produce xT [128, 6, 128] in SBUF."""
            if (b, t) in x_pref:
                xx = x_pref.pop((b, t))
            else:
                xx = p_x.tile([128, D], F32R, tag="x")
                nc.sync.dma_start(xx[:], x_d[b, t * 128:(t + 1) * 128, :])
            xt = p_xt.tile([128, 6, 128], F32R, tag="xt")
            tp0 = ps_t.tile([128, 512], F32R, tag="t")
            for i in range(4):
                transpose_to(tp0[:, i * 128:(i + 1) * 128], xx[:, i * 128:(i + 1) * 128],
                             start=(i == 0), stop=(i == 3))
            nc.vector.tensor_copy(xt[:, 0:4, :].rearrange("p a b -> p (a b)"), _f(tp0[:]))
            tp1 = ps_t.tile([128, 512], F32R, tag="t")
            for i in range(2):
                transpose_to(tp1[:, i * 128:(i + 1) * 128], xx[:, (4 + i) * 128:(5 + i) * 128],
                             start=(i == 0), stop=(i == 1))
            nc.vector.tensor_copy(xt[:, 4:6, :].rearrange("p a b -> p (a b)"),
                                  _f(tp1[:, 0:256]))
            return xt

        cent_store = {}
        inv_sqrt_p = float(1.0 / np.sqrt(np.float32(P)))

        def pass1_tile(b, t):
            if b not in cent_store:
                cent_store[b] = p_cacc.tile([C, GFS], F32R, tag="cacc",
                                            name=f"cacc{b}")
            cent_acc = cent_store[b]
            if True:
                xt = load_xt(b, t)

                # fea = x @ W_exp (+ b_exp)   [128 tok, 1536]
                fea = p_fea.tile([128, EF], F32R, tag="fea")
                for n3 in range(3):
                    fp = ps_mm.tile([128, 512], F32, tag="mm")
                    for k in range(6):
                        nc.tensor.matmul(
                            fp[:], _r(xt[:, k, :]),
                            _r(wexp_sb[:, k, n3 * 512:(n3 + 1) * 512]),
                            start=(k == 0), stop=(k == 5))
                    dst = fea[:, n3 * 512:(n3 + 1) * 512]
                    if has_bexp:
                        nc.vector.tensor_add(dst, fp[:], bexp_sb[:, n3 * 512:(n3 + 1) * 512])
                    else:
                        nc.scalar.copy(dst, fp[:])

                # feaT [128 ef, 12, 128 tok]
                feat = p_feat.tile([128, 12, 128], F32R, tag="feat")
                for grp in range(3):
                    tp = ps_t.tile([128, 512], F32R, tag="t")
                    for i in range(4):
                        m = grp * 4 + i
                        transpose_to(tp[:, i * 128:(i + 1) * 128],
                                     fea[:, m * 128:(m + 1) * 128],
                                     start=(i == 0), stop=(i == 3))
                    nc.vector.tensor_copy(
                        feat[:, grp * 4:(grp + 1) * 4, :].rearrange("p a b -> p (a b)"),
                        _f(tp[:]))

                # act_pre = fea @ cw_f + bias1   [128 tok, 520]
                act = p_act.tile([128, GC + G], F32, tag="act")
                for n2 in range(2):
                    apm = ps_mm.tile([128, 260], F32, tag="mm")
                    for k2 in range(12):
                        nc.tensor.matmul(
                            apm[:], _r(feat[:, k2, :]),
                            _r(cwf_sb[:, k2, n2 * 260:(n2 + 1) * 260]),
                            start=(k2 == 0), stop=(k2 == 11))
                    nc.vector.tensor_add(act[:, n2 * 260:(n2 + 1) * 260], apm[:],
                                         bias1_sb[:, n2 * 260:(n2 + 1) * 260])

                # grouped softmax * sigmoid gate
                e = p_act.tile([128, GC], F32, tag="e")
                nc.scalar.activation(e[:], act[:, 0:GC], AF.Exp)
                ssum = p_sm.tile([128, G], F32, tag="ssum")
                nc.vector.reduce_sum(ssum[:], e[:].rearrange("p (g c) -> p g c", g=G),
                                     axis=mybir.AxisListType.X)
                eneg = p_sm.tile([128, G], F32, tag="eneg")
                nc.scalar.activation(eneg[:], act[:, GC:GC + G], AF.Exp, scale=-1.0)
                nc.vector.tensor_scalar_add(eneg[:], eneg[:], 1.0)
                ga = p_sm.tile([128, G], F32, tag="ga")
                nc.vector.reciprocal(ga[:], eneg[:])
                rs = p_sm.tile([128, G], F32, tag="rs")
                nc.vector.reciprocal(rs[:], ssum[:])
                nc.vector.tensor_mul(rs[:], rs[:], ga[:])
                actf = p_act.tile([128, GC], F32R, tag="actf")
                nc.vector.tensor_tensor(
                    out=actf[:].rearrange("p (g c) -> p g c", g=G),
                    in0=e[:].rearrange("p (g c) -> p g c", g=G),
                    in1=rs[:].unsqueeze(2).broadcast_to((128, G, C)),
                    op=ALU.mult)

                # cent accumulation (diagonal blocks, extracted per tile)
                for mi in range(4):
                    cp = ps_c.tile([128, 384], F32, tag="cent")
                    nc.tensor.matmul(
                        cp[:], _r(actf[:, mi * 128:(mi + 1) * 128]),
                        _r(fea[:, mi * 384:(mi + 1) * 384]),
                        start=True, stop=True)
                    if t == 0 and mi == 0:
                        nc.vector.tensor_copy(cent_acc[:], cp[0:64, 0:192])
                    else:
                        nc.vector.tensor_add(cent_acc[:], cent_acc[:],
                                             cp[0:64, 0:192])
                    nc.vector.tensor_add(cent_acc[:], cent_acc[:],
                                         cp[64:128, 192:384])

        def mid_phase(b):
            cent = cent_store.pop(b)

            # centT [192, 64] as [128, 2, 64]
            centT = mid.tile([128, 2, C], F32R, tag="centT")
            ctp = ps_t.tile([128, 512], F32R, tag="t")
            transpose_to(ctp[:, 0:64], cent[:, 0:128])
            transpose_to(ctp[0:64, 64:128], cent[:, 128:192])
            nc.vector.tensor_copy(centT[:, 0, :], _f(ctp[:, 0:64]))
            nc.vector.tensor_copy(centT[0:64, 1, :], _f(ctp[0:64, 64:128]))

            # nc2 = BN2(cent @ W_proj + b_proj)  [64, 768]
            nc2 = mid.tile([C, D], F32R, tag="nc2")
            for ng, (n0, nn) in enumerate(((0, 512), (512, 256))):
                np_ps = ps_mm.tile([128, 512], F32, tag="mm")
                nc.tensor.matmul(np_ps[0:C, 0:nn], _r(centT[:, 0, :]),
                                 _r(wproj_sb[:, 0, n0:n0 + nn]), start=True, stop=False)
                nc.tensor.matmul(np_ps[0:C, 0:nn], _r(centT[0:64, 1, :]),
                                 _r(wproj_sb[0:64, 1, n0:n0 + nn]), start=False, stop=True)
                nc.vector.scalar_tensor_tensor(
                    out=nc2[:, n0:n0 + nn], in0=np_ps[0:C, 0:nn], scalar=s2_sb[:, 0:1],
                    in1=bias2_sb[:, n0:n0 + nn], op0=ALU.mult, op1=ALU.add)

            # nc2T [768, 64] as [128, 6, 64]
            nc2T = mid.tile([128, 6, C], F32R, tag="nc2T")
            for grp in range(2):
                ntp = ps_t.tile([128, 512], F32R, tag="t")
                for i in range(3):
                    transpose_to(ntp[:, i * 64:(i + 1) * 64],
                                 nc2[:, (grp * 3 + i) * 128:(grp * 3 + i + 1) * 128],
                                 start=(i == 0), stop=(i == 2))
                nc.vector.tensor_copy(
                    nc2T[:, grp * 3:(grp + 1) * 3, :].rearrange("p a b -> p (a b)"),
                    _f(ntp[:, 0:192]))

            # kv = nc2 @ Wkv (+ bkv)   [64, 768]
            kv = mid.tile([C, 2 * P], F32R, tag="kv")
            for ng, (n0, nn) in enumerate(((0, 512), (512, 256))):
                kv_ps = ps_mm.tile([128, 512], F32, tag="mm")
                for k in range(6):
                    nc.tensor.matmul(kv_ps[0:C, 0:nn], _r(nc2T[:, k, :]),
                                     _r(wkv_sb[:, k, n0:n0 + nn]),
                                     start=(k == 0), stop=(k == 5))
                if has_bkv:
                    nc.vector.tensor_add(kv[:, n0:n0 + nn], kv_ps[0:C, 0:nn],
                                         bkv_sb[:, n0:n0 + nn])
                else:
                    nc.scalar.copy(kv[:, n0:n0 + nn], kv_ps[0:C, 0:nn])

            # kT, vT  [128, 3, 64]
            kT = mid.tile([128, 3, C], F32R, tag="kT")
            vT = mid.tile([128, 3, C], F32R, tag="vT")
            ktp = ps_t.tile([128, 512], F32R, tag="t")
            for i in range(3):
                transpose_to(ktp[:, i * 64:(i + 1) * 64], kv[:, i * 128:(i + 1) * 128],
                             start=(i == 0), stop=(i == 2))
            nc.vector.tensor_copy(kT[:].rearrange("p a b -> p (a b)"), _f(ktp[:, 0:192]))
            vtp = ps_t.tile([128, 512], F32R, tag="t")
            for i in range(3):
                transpose_to(vtp[:, i * 64:(i + 1) * 64],
                             kv[:, P + i * 128:P + (i + 1) * 128],
                             start=(i == 0), stop=(i == 2))
            nc.vector.tensor_copy(vT[:].rearrange("p a b -> p (a b)"), _f(vtp[:, 0:192]))

            # wqk = Wq @ k^T  [768, 64] as [128, 6, 64]
            wqk = mid.tile([128, 6, C], F32R, tag="wqk")
            for m in range(6):
                wq_ps = ps_t.tile([128, 512], F32, tag="t")
                for k3 in range(3):
                    nc.tensor.matmul(wq_ps[:, 0:C], _r(wqT_sb[:, k3, m * 128:(m + 1) * 128]),
                                     _r(kT[:, k3, :]), start=(k3 == 0), stop=(k3 == 2))
                nc.scalar.copy(wqk[:, m, :], wq_ps[:, 0:C])
            if has_bq:
                bc_ps = ps_t.tile([128, 512], F32, tag="t")
                for k3 in range(3):
                    nc.tensor.matmul(bc_ps[0:1, 0:C], _r(bq_sb[:, k3, :]),
                                     _r(kT[:, k3, :]), start=(k3 == 0), stop=(k3 == 2))
                bias_c = mid.tile([1, C], F32R, tag="bias_c")
                nc.scalar.copy(bias_c[:], bc_ps[0:1, 0:C])

            if not has_bq:
                bias_c = None

            # vW = v @ Wp2  [64, 768]
            vw = mid.tile([C, D], F32R, tag="vw")
            for ng, (n0, nn) in enumerate(((0, 512), (512, 256))):
                vw_ps = ps_mm.tile([128, 512], F32, tag="mm")
                for k3 in range(3):
                    nc.tensor.matmul(vw_ps[0:C, 0:nn], _r(vT[:, k3, :]),
                                     _r(wp2_sb[:, k3, n0:n0 + nn]),
                                     start=(k3 == 0), stop=(k3 == 2))
                nc.scalar.copy(vw[:, n0:n0 + nn], vw_ps[0:C, 0:nn])

            return {"wqk": wqk, "vw": vw,
                    "bias_c": bias_c}

        def pass2_tile(b, t, mt):
            wqk, vw, bias_c = mt["wqk"], mt["vw"], mt["bias_c"]
            if True:
                xt = load_xt(b, t)
                sc_ps = ps_t.tile([128, 512], F32, tag="t")
                for k in range(6):
                    nc.tensor.matmul(sc_ps[:, 0:C], _r(xt[:, k, :]), _r(wqk[:, k, :]),
                                     start=(k == 0), stop=(k == 5 and not has_bq),
                                     skip_group_check=True)
                if has_bq:
                    nc.tensor.matmul(sc_ps[:, 0:C], _r(ones_sb[:]), _r(bias_c[:]),
                                     start=False, stop=True, skip_group_check=True)

                e_att = p_sm.tile([128, C], F32, tag="e_att")
                ssum_a = p_sm.tile([128, 1], F32, tag="ssum_a")
                nc.scalar.activation(e_att[:], sc_ps[:, 0:C], AF.Exp,
                                     scale=inv_sqrt_p, accum_out=ssum_a[:])
                rs_a = p_sm.tile([128, 1], F32, tag="rs_a")
                nc.vector.reciprocal(rs_a[:], ssum_a[:])
                attn = p_sm.tile([128, C], F32R, tag="attn")
                nc.vector.tensor_scalar_mul(attn[:], e_att[:], rs_a[:])

                at_ps = ps_t.tile([128, 512], F32R, tag="t")
                transpose_to(at_ps[0:C, 0:128], attn[:])
                attnT = p_sm.tile([C, 128], F32R, tag="attnT")
                nc.vector.tensor_copy(attnT[:], _f(at_ps[0:C, 0:128]))

                outt = p_out.tile([128, D], F32, tag="outt")
                for ng, (n0, nn) in enumerate(((0, 512), (512, 256))):
                    fo_ps = ps_mm.tile([128, 512], F32, tag="mm")
                    nc.tensor.matmul(fo_ps[:, 0:nn], _r(attnT[:]),
                                     _r(vw[:, n0:n0 + nn]), start=True, stop=True)
                    if has_bp2:
                        nc.vector.tensor_add(outt[:, n0:n0 + nn], fo_ps[:, 0:nn],
                                             bp2_sb[:, n0:n0 + nn])
                    else:
                        nc.scalar.copy(outt[:, n0:n0 + nn], fo_ps[:, 0:nn])
                nc.sync.dma_start(out_d[b, t * 128:(t + 1) * 128, :], outt[:])

        # ---------------- interleaved emission ----------------
        # pass1 of batch b+1 overlaps mid+pass2 of batch b so the PE never
        # idles long enough to re-trigger the HAM throttle.
        for t in range(NT):
            pass1_tile(0, t)
        for b in range(NB):
            nxt = b + 1 if b + 1 < NB else None
            if nxt is not None:
                pass1_tile(nxt, 0)
                pass1_tile(nxt, 1)
            mt = mid_phase(b)
            for t in range(NT):
                pass2_tile(b, t, mt)
                if nxt is not None and t + 2 < NT:
                    pass1_tile(nxt, t + 2)

    nc.compile()
    return nc


_PROGRAM_CACHE = {}


def _prep(inputs):
    """Host-side folds. Returns (flags, in_map_common)."""
    f32 = np.float32
    g = {k: np.ascontiguousarray(np.asarray(v, dtype=f32)) for k, v in inputs.items()}
    s1 = g["bn1_g"] / np.sqrt(g["bn1_v"] + f32(EPS))
    cwf = np.concatenate([g["cluster_weights"] * s1[None, :], g["W_ga"]], axis=1)
    bias1 = np.concatenate([g["bn1_b"] - g["bn1_m"] * s1, g["b_ga"]])
    s2 = g["bn2_g"] / np.sqrt(g["bn2_v"] + f32(EPS))
    bias2 = (g["b_proj"][None, :] - g["bn2_m"][:, None]) * s2[:, None] + g["bn2_b"][:, None]
    flags = (
        bool(np.any(g["b_exp"])),
        bool(np.any(g["bq"])),
        bool(np.any(g["bkv"])),
        bool(np.any(g["bp2"])),
    )
    common = {
        "wexp": g["W_exp"],
        "cwf": np.ascontiguousarray(cwf),
        "bias1": np.ascontiguousarray(bias1),
        "wproj": g["W_proj"],
        "s2": np.ascontiguousarray(s2.reshape(C, 1)),
        "bias2": np.ascontiguousarray(bias2),
        "wkv": g["Wkv"],
        "wqT": np.ascontiguousarray(g["Wq"].T),
        "wp2": g["Wp2"],
    }
    if flags[0]:
        common["bexp"] = g["b_exp"]
    if flags[1]:
        common["bqT"] = np.ascontiguousarray(g["bq"].reshape(P, 1))
    if flags[2]:
        common["bkv"] = g["bkv"]
    if flags[3]:
        common["bp2"] = g["bp2"]
    return flags, common, g["x"]


def run(inputs, trace=False):
    flags, common, x = _prep(inputs)
    if flags not in _PROGRAM_CACHE:
        _PROGRAM_CACHE[flags] = build_program(flags)
    nc = _PROGRAM_CACHE[flags]
    in_maps = []
    for c in range(N_CORES):
        m = dict(common)
        m["x"] = np.ascontiguousarray(x[c * NB:(c + 1) * NB])
        in_maps.append(m)
    res = bass_utils.run_bass_kernel_spmd(
        nc, in_maps, core_ids=list(range(N_CORES)), trace=trace)
    out = np.concatenate([r["out"] for r in res.results], axis=0)
    return out, res


def kernel(**inputs):
    out, _ = run(inputs, trace=False)
    return out
